# revision 1
# baseline (speedup 1.0000x reference)
"""Causal self-attention (B=2, T=2048, C=1024, H=16) on 8 TRN2 NeuronCores.

Sharding: core c -> batch b = c//4, head-group g = c%4 (4 heads = 256 channels).
Each core computes its 4 heads end-to-end and a partial projection
(y_local @ W_proj[256g:256g+256, :]); the host sums the 4 partials per batch.

v2 dataflow (cost-model-driven; 111450 ns vs the 160527 ns fp32r baseline):
  - QKV/V matmuls in fp8e4m3 DoubleRow with two-term compensation:
    (x8+xr8)@(w8+wr8) dropping xr8@wr8 -> 12 DR passes per chunk at
    0.5 cyc/col (0.75x the fp32r cost), weights pre-scaled x64 on host
    (descaled in the psum->sbuf move) to clear fp8's subnormal floor.
  - S^T = k_h^T q_h per head in f16, exact 128-granular causal windows.
    Both heads of a pair share one [128,2,512] psum tile; one fused exp
    per chunk on ACT.
  - Causal masking via 0/1 f16 multiply on DVE (2x mode) after exp --
    no mask matmuls on PE.
  - AV in natural [q,d] layout: lhsT = pt q-slices (stationary loads are
    free in the cost model), rhs = [1|v_h] -> 65-col matmuls, half the
    transposed-layout cost. Softmax denominator rides column 0; the
    normalize is a reciprocal + per-partition tensor_scalar multiply.
  - Normalized y transposed back via PE is_transpose (f16), then f16 proj.
  - Output f16; host upcasts, sums partials, adds b_proj.

Scheduling: engines run their streams in emission order. The exp chain on
ACT is the scarce resource, so AV runs software-pipelined `depth` chunks
behind S/exp (deep pt pool), the next round's first S/exp chunk is
pre-rolled before each AV tail, per-q-subtile normalize/transpose stream
out as their accumulation groups close, and qkv/v waves + projections are
metered into the chunk slots as fillers (pair-1 wave chunks deliberately
late, a budget holds projs back for the last round). The DMA ramp ships
combined main||resid fp8 tensors with warm matmuls soaking up arrival
gaps; steady-state outputs go out over HWDGE, tail outputs alternate
HWDGE/SWDGE.
"""

import numpy as np

B, T, C = 2, 2048, 1024
H, HD = 16, 64
NCORES = 8
HEADS_PER_CORE = 4          # 2 pairs
CH = HEADS_PER_CORE * HD    # 256 channels per core
KT8 = 4                     # fp8 contraction pair-tiles (K=256 each)
NT = T // 128               # 16 t tiles
NJ = T // 512               # 4 query chunks
SCALE = 1.0 / np.sqrt(HD)
WS = 64.0                   # host-side weight pre-scale for fp8 range

_COMPILED = None


def _build():
    import concourse.bass as bass
    import concourse.bacc as bacc
    import concourse.mybir as mybir
    import concourse.tile as tile

    f32 = mybir.dt.float32
    f16 = mybir.dt.float16
    f8 = mybir.dt.float8e4
    DR = mybir.MatmulPerfMode.DoubleRow
    Exp = mybir.ActivationFunctionType.Exp
    mult = mybir.AluOpType.mult
    add = mybir.AluOpType.add
    div = mybir.AluOpType.divide

    nc = bacc.Bacc("TRN2", target_bir_lowering=False, debug=False)

    # combined main||residual fp8 tensors: one DMA feeds both comp8 terms
    x8_d = nc.dram_tensor("x8c", [KT8, 128, 2, 2, T], f8, kind="ExternalInput").ap()
    w8_d = nc.dram_tensor("w8c", [KT8, 128, 2, 2 * 3 * CH], f8, kind="ExternalInput").ap()
    wp_d = nc.dram_tensor("wp", [2, 128, C], f16, kind="ExternalInput").ap()
    bqk_d = nc.dram_tensor("bqk", [128, 4], f32, kind="ExternalInput").ap()
    bv_d = nc.dram_tensor("bv", [1, CH], f32, kind="ExternalInput").ap()
    tri_d = nc.dram_tensor("tri", [128, 128], f16, kind="ExternalInput").ap()
    ident_d = nc.dram_tensor("ident", [128, 128], f16, kind="ExternalInput").ap()
    out_d = nc.dram_tensor("out_p", [T, C], f16, kind="ExternalOutput").ap()

    with tile.TileContext(nc) as tc:
        with (
            tc.tile_pool(name="p_w", bufs=1) as p_w,
            tc.tile_pool(name="p_x", bufs=1) as p_x,
            tc.tile_pool(name="p_qk", bufs=1) as p_qk,
            tc.tile_pool(name="p_v", bufs=1) as p_v,
            tc.tile_pool(name="p_y", bufs=1) as p_y,
            tc.tile_pool(name="p_pt", bufs=12) as p_pt,
            tc.tile_pool(name="p_yn", bufs=16) as p_yn,
            tc.tile_pool(name="p_st", bufs=4) as p_st,
            tc.tile_pool(name="ps_s", bufs=2, space="PSUM") as ps_s,
            tc.tile_pool(name="ps_y", bufs=1, space="PSUM") as ps_y,
            tc.tile_pool(name="ps_mm", bufs=2, space="PSUM") as ps_mm,
        ):
            # ---- persistent inputs -------------------------------------
            ident = p_w.tile([128, 128], f16, name="ident", tag="ident")
            tri = p_w.tile([128, 128], f16, name="tri", tag="tri")
            bqk = p_w.tile([128, 4], f32, name="bqk", tag="bqk")
            bvrow = p_w.tile([1, CH], f32, name="bvrow", tag="bvrow")
            bvb = p_w.tile([128, CH], f32, name="bvb", tag="bvb")
            w8c = [p_w.tile([128, 2, 2 * 3 * CH], f8, name=f"w8_{k}", tag=f"w8_{k}")
                   for k in range(KT8)]
            x8c = [p_x.tile([128, 2, 2, T], f8, name=f"x8_{k}", tag=f"x8_{k}")
                   for k in range(KT8)]
            wp = [p_w.tile([128, C], f16, name=f"wp{k}", tag=f"wp{k}")
                  for k in range(2)]
            # views into the [qk-main|qk-resid|v-main|v-resid] column layout
            w8qk = [w8c[k][:, :, 0:512] for k in range(KT8)]
            wr8qk = [w8c[k][:, :, 512:1024] for k in range(KT8)]
            w8v = [w8c[k][:, :, 1024:1280] for k in range(KT8)]
            wr8v = [w8c[k][:, :, 1280:1536] for k in range(KT8)]
            x8 = [x8c[k][:, :, 0, :] for k in range(KT8)]
            xr8 = [x8c[k][:, :, 1, :] for k in range(KT8)]

            # warmup sources first (Pool memset, no deps) so PE can spin
            wsrc = p_w.tile([128, 128], f16, name="wsrc", tag="wsrc")
            nc.gpsimd.memset(wsrc, 0.5)
            wsrc2 = p_w.tile([128, 512], f16, name="wsrc2", tag="wsrc2")
            nc.gpsimd.memset(wsrc2, 0.5)
            # small consts via SWDGE (Pool) so HWDGE is free for the bulk ramp
            nc.gpsimd.dma_start(out=bqk, in_=bqk_d)
            nc.gpsimd.dma_start(out=bvrow, in_=bv_d)
            nc.gpsimd.dma_start(out=tri, in_=tri_d)
            nc.gpsimd.dma_start(out=ident, in_=ident_d)
            nc.gpsimd.partition_broadcast(bvb, bvrow[0:1, :])
            # bulk ramp: per k, first x t-quarter + comp8 qk weights (unlocks
            # the S side); v weights follow, then the remaining x quarters
            for k in range(KT8):
                nc.sync.dma_start(out=x8c[k][:, :, :, 0:512],
                                  in_=x8_d[k][:, :, :, 0:512])
                nc.sync.dma_start(out=w8c[k][:, :, 0:1024],
                                  in_=w8_d[k][:, :, 0:1024])
            for k in range(KT8):
                nc.sync.dma_start(out=w8c[k][:, :, 1024:1536],
                                  in_=w8_d[k][:, :, 1024:1536])
            for q in (1, 2, 3):
                for k in range(KT8):
                    nc.sync.dma_start(
                        out=x8c[k][:, :, :, 512 * q:512 * (q + 1)],
                        in_=x8_d[k][:, :, :, 512 * q:512 * (q + 1)])
            for k in range(2):
                nc.sync.dma_start(out=wp[k], in_=wp_d[k])

            # ---- persistent intermediates ------------------------------
            # qT/kT tile p: head pair p, heads (2p, 2p+1) on partitions 0:64/64:128
            qT = [p_qk.tile([128, T], f16, name=f"qT{p}", tag=f"qT{p}") for p in range(2)]
            kT = [p_qk.tile([128, T], f16, name=f"kT{p}", tag=f"kT{p}") for p in range(2)]
            # v tiles: [128 t, 4 heads, 65] -- col 0 of each head = 1.0 (denominator)
            v = [p_v.tile([128, 4, 65], f16, name=f"v{m}", tag=f"v{m}") for m in range(NT)]
            # normalized y^T per pair: [128 ch, T]
            ynT = [p_y.tile([128, T], f16, name=f"ynT{p}", tag=f"ynT{p}") for p in range(2)]

            # PE p-state warmup: cheap dependency-light matmuls. Target lives
            # in the y-pool, which nothing touches until attention(0,0)'s
            # first AV (well after the ramp).
            warm = ps_y.tile([128, 512], f32, name="warm", tag="y0")

            def warm_mms(n):
                for _ in range(n):
                    nc.tensor.matmul(warm, lhsT=wsrc, rhs=wsrc2,
                                     start=True, stop=True)
            # first spins only need the small (fast) memset source
            for _ in range(4):
                nc.tensor.matmul(warm[:, 0:128], lhsT=wsrc, rhs=wsrc,
                                 start=True, stop=True)
            warm_mms(7)

            GROUPS = ((x8, w8qk, w8v), (x8, wr8qk, wr8v), (xr8, w8qk, w8v))

            def qkv_mms(ps, mi, nj, glist):
                for g in glist:
                    xa, wb, _ = GROUPS[g]
                    for kk in range(KT8):
                        nc.tensor.matmul(
                            ps[:, 0:512],
                            lhsT=wb[kk][:, :, 128 * mi:128 * (mi + 1)],
                            rhs=xa[kk][:, :, 512 * nj:512 * (nj + 1)],
                            start=(g == 0 and kk == 0),
                            stop=(g == 2 and kk == KT8 - 1),
                            perf_mode=DR,
                        )

            def qkv_move(ps, mi, nj):
                dst = (qT if mi < 2 else kT)[mi % 2][:, 512 * nj:512 * (nj + 1)]
                nc.vector.tensor_scalar(dst, ps[:, 0:512], 1.0 / WS,
                                        bqk[:, mi:mi + 1], mult, add)

            def qkv_chunk(mi, nj):
                """q/k channels [128mi,128mi+128), t [512nj, 512nj+512)."""
                ps = ps_mm.tile([128, 512], f32, name="ps_qkv", tag="mm")
                qkv_mms(ps, mi, nj, (0, 1, 2))
                qkv_move(ps, mi, nj)

            def qkv_pieces(mi, nj):
                """qkv chunk as 3 filler pieces (~0.43us each) so per-slot
                PE filler mass matches the exp-pacing deficit."""
                st = {}

                def p0():
                    st["ps"] = ps_mm.tile([128, 512], f32, name="ps_qkv",
                                          tag="mm")
                    qkv_mms(st["ps"], mi, nj, (0,))
                    qkv_mms(st["ps"], mi, nj, (1,))
                return [p0,
                        lambda: (qkv_mms(st["ps"], mi, nj, (2,)),
                                 qkv_move(st["ps"], mi, nj))]

            def v_pieces(m):
                st = {}

                def p0():
                    st["ps"] = ps_mm.tile([128, 512], f32, name="ps_v",
                                          tag="mm")
                    v_mms(st["ps"], m, (0, 1))
                return [p0,
                        lambda: (v_mms(st["ps"], m, (2,)), v_move(st["ps"], m))]

            def proj_pieces(m):
                st = {"st": None}

                def pu(u):
                    if u == 0:
                        st["st"] = p_st.tile([128, 1024], f16, name="st_pr",
                                             tag="st")
                    ps = ps_mm.tile([128, 512], f32, name="ps_pr", tag="mm")
                    for kk in range(2):
                        nc.tensor.matmul(
                            ps[:, 0:512],
                            lhsT=ynT[kk][:, 128 * m:128 * (m + 1)],
                            rhs=wp[kk][:, 512 * u:512 * (u + 1)],
                            start=(kk == 0), stop=(kk == 1),
                        )
                    nc.vector.tensor_copy(
                        st["st"][:, 512 * u:512 * (u + 1)], ps[:, 0:512])
                    if u == 1:
                        nc.sync.dma_start(
                            out=out_d[128 * m:128 * (m + 1), :], in_=st["st"])
                return [lambda: pu(0), lambda: pu(1)]

            def v_mms(ps, m, glist):
                for g in glist:
                    xa, _, wb = GROUPS[g]
                    for kk in range(KT8):
                        nc.tensor.matmul(
                            ps[:, 0:CH],
                            lhsT=xa[kk][:, :, 128 * m:128 * (m + 1)],
                            rhs=wb[kk],
                            start=(g == 0 and kk == 0),
                            stop=(g == 2 and kk == KT8 - 1),
                            perf_mode=DR,
                        )

            def v_move(ps, m):
                nc.vector.memset(v[m][:, :, 0:1], 1.0)
                nc.vector.scalar_tensor_tensor(
                    v[m][:, :, 1:65],
                    ps[:, 0:CH].rearrange("p (h c) -> p h c", h=4),
                    1.0 / WS,
                    bvb.rearrange("p (h c) -> p h c", h=4),
                    mult, add,
                )

            def v_chunk(m):
                """v rows [128m, 128m+128), all 4 heads."""
                ps = ps_mm.tile([128, 512], f32, name="ps_v", tag="mm")
                v_mms(ps, m, (0, 1, 2))
                v_move(ps, m)

            prerolled = {}

            def pool_exp(out, in_):
                """exp on the gpsimd engine: parallel pt-production queue."""
                eng = nc.gpsimd
                imm = lambda val: mybir.ImmediateValue(dtype=f32, value=val)
                eng.add_instruction(mybir.InstActivation(
                    name=eng.bass.get_next_instruction_name(),
                    func=Exp,
                    ins=[eng.lower_ap(in_), imm(0.0), imm(1.0), imm(0.0)],
                    outs=[eng.lower_ap(out)],
                ))

            def s_exp_chunk(j, p, i):
                """S matmuls + fused exp (+ diag tri-mask) for one chunk."""
                rr = i - 4 * j
                W0 = 128 * rr if rr > 0 else 0
                s2 = ps_s.tile([128, 2, 512], f32, name="s2", tag="s")
                for h in range(2):
                    nc.tensor.matmul(
                        s2[:, h, W0:512],
                        lhsT=kT[p][64 * h:64 * h + 64, 128 * i:128 * (i + 1)],
                        rhs=qT[p][64 * h:64 * h + 64, 512 * j + W0:512 * (j + 1)],
                        start=True, stop=True,
                    )
                pt = p_pt.tile([128, 2, 512], f16, name="pt", tag="pt")
                if False:
                    pool_exp(pt[:, :, W0:512], s2[:, :, W0:512])
                else:
                    nc.scalar.activation(pt[:, :, W0:512], s2[:, :, W0:512], Exp)
                if rr >= 0:
                    for h in range(2):
                        nc.vector.tensor_tensor(
                            pt[:, h, W0:W0 + 128], pt[:, h, W0:W0 + 128],
                            tri, mult)
                return pt

            def attention(j, p, filler=None, depth=2, tail_hook=None,
                          next_jp=None, preroll_drain=None):
                """q-chunk j (512 queries), head pair p (heads 2p, 2p+1).
                AV is software-pipelined `depth` chunks behind S/exp so the
                PE stream never parks on the exp it just requested. The next
                round's first S/exp chunk is pre-rolled before the AV tail to
                hide the s2-pool rotation wait at the round boundary."""
                ni = 4 * j + 4
                yp = [ps_y.tile([128, 4, 65], f32, name=f"y{h}", tag=f"y{h}")
                      for h in range(2)]
                pts = prerolled.pop((j, p), {})

                deferred = []

                def av(i):
                    rr = i - 4 * j
                    pt = pts.pop(i)
                    for h in range(2):
                        for tt in range(max(0, rr), 4):
                            nc.tensor.matmul(
                                yp[h][:, tt, 0:65],
                                lhsT=pt[:, h, 128 * tt:128 * (tt + 1)],
                                rhs=v[i][:, 2 * p + h, :],
                                start=(i == 0 and tt == 0),
                                stop=(i == 4 * j + tt),
                                skip_group_check=True,
                            )
                    # q-subtile tt's accumulation closed at chunk 4j+tt:
                    # stream its normalize (+ tail work) immediately
                    tt = i - 4 * j
                    if tt >= 0:
                        norm(tt)

                def norm(tt):
                    # ISA TensorScalar has no divide: per-head reciprocal of
                    # the denominator (col 0), then scalar-multiply
                    rc = p_yn.tile([128, 2], f32, name="rc", tag="rc", bufs=8)
                    ynst = p_yn.tile([128, 128], f16, name="ynst", tag="yn")
                    for h in range(2):
                        nc.vector.reciprocal(rc[:, h:h + 1], yp[h][:, tt, 0:1])
                        nc.vector.tensor_scalar(
                            ynst[:, 64 * h:64 * h + 64],
                            yp[h][:, tt, 1:65], rc[:, h:h + 1], None, mult)

                    def transpose_move(tt=tt, ynst=ynst):
                        tp = ps_mm.tile([128, 128], f16, name="tp", tag="mm")
                        nc.tensor.transpose(tp, ynst, ident)
                        nc.vector.tensor_copy(
                            ynT[p][:, 512 * j + 128 * tt:
                                   512 * j + 128 * (tt + 1)], tp)
                    if tail_hook is not None:
                        transpose_move()
                        tail_hook(tt)
                    else:
                        deferred.append(transpose_move)

                for i in range(ni):
                    if i not in pts:
                        pts[i] = s_exp_chunk(j, p, i)
                    if i >= depth:
                        av(i - depth)
                    if filler is not None:
                        filler()
                if next_jp is not None:
                    # the next round reads qT/kT written by wave fillers --
                    # force the relevant ones out first
                    preroll_drain(next_jp)
                    prerolled[next_jp] = {0: s_exp_chunk(*next_jp, 0)}
                for i in range(ni - depth, ni):
                    av(i)
                return deferred

            def proj(m, tail=False):
                """output rows [128m, 128m+128): 2 c-halves into one staging tile.
                Steady state: staging on Pool, DMA via SWDGE (keeps DVE/HWDGE
                free). Tail: parallel DVE+ACT staging, half-DMAs via HWDGE."""
                st = p_st.tile([128, 1024], f16, name="st_pr", tag="st")
                for u in range(2):
                    if tail and u == 1:
                        # S pool is idle in the tail; avoids mm-slot waits
                        ps = ps_s.tile([128, 512], f32, name="ps_prs", tag="s")
                    else:
                        ps = ps_mm.tile([128, 512], f32, name="ps_pr", tag="mm")
                    for kk in range(2):
                        nc.tensor.matmul(
                            ps[:, 0:512],
                            lhsT=ynT[kk][:, 128 * m:128 * (m + 1)],
                            rhs=wp[kk][:, 512 * u:512 * (u + 1)],
                            start=(kk == 0), stop=(kk == 1),
                        )
                    stu = st[:, 512 * u:512 * (u + 1)]
                    if tail and u == 1:
                        nc.scalar.copy(stu, ps[:, 0:512])
                    else:
                        nc.vector.tensor_copy(stu, ps[:, 0:512])
                eng = nc.gpsimd if (tail and m in (12, 13)) else nc.sync
                eng.dma_start(out=out_d[128 * m:128 * (m + 1), :], in_=st)

            # ---- emission order (scheduling priority) -------------------
            # ramp: wave 0, first two chunks split so the fp8 main group runs
            # as soon as w8/x8 land; residuals follow when wr8/xr8 arrive.
            # the kk-interleaved ramp: warm matmuls (no deps) soak up the
            # per-DMA arrival gaps of the [x8q1[k], w8qk[k]] stream.
            ps_a = ps_mm.tile([128, 512], f32, name="ps_qkv", tag="mm")
            ps_b = ps_mm.tile([128, 512], f32, name="ps_qkv", tag="mm")
            for kk in range(KT8):
                for ps, mi in ((ps_a, 0), (ps_b, 2)):
                    nc.tensor.matmul(
                        ps[:, 0:512],
                        lhsT=w8qk[kk][:, :, 128 * mi:128 * (mi + 1)],
                        rhs=x8[kk][:, :, 0:512],
                        start=(kk == 0), stop=False, perf_mode=DR,
                    )
                warm_mms(4)
            qkv_mms(ps_a, 0, 0, (1, 2))
            qkv_move(ps_a, 0, 0)
            qkv_mms(ps_b, 2, 0, (1, 2))
            qkv_move(ps_b, 2, 0)
            for mi in (1, 3):
                qkv_chunk(mi, 0)
            # v(0..3) ride as the first fillers of attention(0,0): their
            # x/w data lands after the qk stream, and j=0 runs AV depth-4
            # so no AV precedes them.

            # waves: pair-0 qkv of the next j -- must emit before attention
            # (j+1, 0) (drained at the j boundary / cross-j preroll).
            # waves_late: pair-1 qkv -- only read by (j+1, 1); they fill
            # (j+1, 0)'s ACT-paced slots and drain at the (j+1,1) preroll.
            # v chunks of wave w front-fill (w, 0): av(m) runs late in its
            # own round. ordered: transposes + projs -- span boundaries.
            waves = []
            waves_late = []
            ordered = []
            budget = [None]  # per-round cap on consumed `ordered` fillers

            def filler():
                if waves:
                    waves.pop(0)()
                elif waves_late:
                    waves_late.pop(0)()
                elif ordered:
                    if budget[0] is not None:
                        if budget[0] <= 0:
                            return
                        budget[0] -= 1
                    ordered.pop(0)()

            def drain(lst):
                while lst:
                    lst.pop(0)()

            vfront = {w: [lambda m=m: v_chunk(m)
                          for m in range(4 * w, 4 * w + 4)]
                      for w in range(4)}

            def preroll_drain(next_jp):
                if next_jp[1] == 1:
                    drain(waves_late)
                else:
                    drain(waves)

            # phase order front-loads the big j=3 round right after its data
            # lands, so ACT saturates early and the later (smaller) rounds
            # swim in proj/wave filler mass.
            # causality: attention(j) reads kT columns of ALL waves <= j, so
            # rounds must run in j order.
            for j in range(NJ):
                if j < 3:
                    nxt_w = j + 1
                    for mi in (0, 2):
                        waves.append(lambda mi=mi, nj=nxt_w: qkv_chunk(mi, nj))
                    for mi in (1, 3):
                        waves_late.append(lambda mi=mi, nj=nxt_w: qkv_chunk(mi, nj))
                # earlier t-blocks' projections, kept late to feed PE while
                # ACT drains the (larger) late-j exp queue
                if j == 2:
                    for m in range(0, 4):
                        ordered.append(lambda m=m: proj(m))
                elif j == 3:
                    for m in range(4, 12):
                        ordered.append(lambda m=m: proj(m))
                for p in range(2):
                    if p == 0:
                        waves[0:0] = vfront.pop(j)
                    budget[0] = 4 if (j, p) == (3, 0) else None
                    nxt = (j, 1) if p == 0 else ((j + 1, 0) if j < 3 else None)
                    if (j, p) == (3, 1):
                        def tail_hook(tt):
                            drain(ordered)
                            proj(12 + tt, tail=True)
                        attention(j, p, filler, tail_hook=tail_hook)
                    else:
                        deferred = attention(j, p, filler,
                                             depth=(4 if j == 0 else 8),
                                             next_jp=nxt,
                                             preroll_drain=preroll_drain)
                        ordered.extend(deferred)
                drain(waves)

    nc.compile()
    return nc


def _host_inputs(x, W_attn, b_attn, W_proj):
    """Build the 8 per-core input maps (numpy only)."""
    import ml_dtypes
    f8 = ml_dtypes.float8_e4m3

    x = np.asarray(x, dtype=np.float32)
    W_attn = np.asarray(W_attn, dtype=np.float32)
    b_attn = np.asarray(b_attn, dtype=np.float32)
    W_proj = np.asarray(W_proj, dtype=np.float32)

    # strict causal 0/1 mask for the 128x128 diagonal blocks: valid iff c >= k
    kl = np.arange(128)
    tri = (kl[None, :] >= kl[:, None]).astype(np.float16)
    ident = np.eye(128, dtype=np.float16)

    def pack8(a):
        """[C, N] -> fp8 main/residual tiles [KT8, 128, 2, N] each."""
        a8 = a.astype(f8)
        ar8 = (a - a8.astype(np.float32)).astype(f8)
        def t(z):
            return z.reshape(KT8, 2, 128, a.shape[1]).transpose(0, 2, 1, 3)
        return t(a8), t(ar8)

    in_maps = []
    for c in range(NCORES):
        b, g = divmod(c, 4)
        sl = slice(CH * g, CH * (g + 1))
        wq = W_attn[:, 0 * C:1 * C][:, sl] * SCALE
        wk = W_attn[:, 1 * C:2 * C][:, sl]
        wv = W_attn[:, 2 * C:3 * C][:, sl]
        bq = b_attn[0 * C:1 * C][sl] * SCALE
        bk = b_attn[1 * C:2 * C][sl]
        bv = b_attn[2 * C:3 * C][sl]
        bqk = np.stack([bq[0:128], bq[128:256], bk[0:128], bk[128:256]], axis=1)
        wfull = np.concatenate([wq, wk, wv], axis=1) * WS     # [1024, 768]
        w8, wr8 = pack8(wfull)
        xT = np.ascontiguousarray(x[b].T)                     # [1024, 2048]
        x8, xr8 = pack8(xT)
        # columns: [qk-main | qk-resid | v-main | v-resid]
        w8c = np.ascontiguousarray(np.concatenate(
            [w8[..., 0:512], wr8[..., 0:512],
             w8[..., 512:768], wr8[..., 512:768]], axis=3))
        x8c = np.ascontiguousarray(np.stack([x8, xr8], axis=3))
        in_maps.append({
            "x8c": x8c, "w8c": w8c,
            "wp": np.ascontiguousarray(
                W_proj[sl, :].reshape(2, 128, C).astype(np.float16)),
            "bqk": np.ascontiguousarray(bqk),
            "bv": np.ascontiguousarray(bv[None, :]),
            "tri": tri, "ident": ident,
        })
    return in_maps


def kernel(x, W_attn, b_attn, W_proj, b_proj, _want_results=None):
    global _COMPILED
    from concourse.bass_utils import run_bass_kernel_spmd

    if _COMPILED is None:
        _COMPILED = _build()
    nc = _COMPILED

    in_maps = _host_inputs(x, W_attn, b_attn, W_proj)
    kw = dict(_want_results or {})
    res = run_bass_kernel_spmd(nc, in_maps, core_ids=list(range(NCORES)), **kw)
    if _want_results is not None:
        kernel.last_results = res

    out = np.zeros((B, T, C), dtype=np.float32)
    for c in range(NCORES):
        out[c // 4] += res.results[c]["out_p"].astype(np.float32)
    out += np.asarray(b_proj, dtype=np.float32)[None, None, :]
    return out



# revision 45
# speedup vs baseline: 1.0156x; 1.0156x over previous
"""Causal self-attention (B=2, T=2048, C=1024, H=16) on 8 TRN2 NeuronCores.

Sharding: core c -> batch b = c//4, head-group g = c%4 (4 heads = 256 channels).
Each core computes its 4 heads end-to-end and a partial projection
(y_local @ W_proj[256g:256g+256, :]); the host sums the 4 partials per batch.

v3 dataflow (cost-model-driven; v2 was 111450 ns):
  - QKV/V matmuls in fp8e4m3 DoubleRow with two-term compensation:
    (x8+xr8)@(w8+wr8) dropping xr8@wr8 -> 12 DR passes per chunk at
    0.5 cyc/col, weights pre-scaled x64 on host (descaled in the
    psum->sbuf move) to clear fp8's subnormal floor.
  - S^T = k_h^T q_h per head in f16, exact 128-granular causal windows.
    Both heads of a pair share one [128,2,512] psum tile; one fused exp
    per chunk on ACT.  Causal masking via 0/1 f16 multiply on DVE after
    exp.
  - AV in natural [q,d] layout: lhsT = pt q-slices, rhs = [1|v_h] ->
    65-col matmuls; softmax denominator rides column 0; normalize is a
    reciprocal + per-partition tensor_scalar multiply.
  - Normalized y transposed back via PE is_transpose (f16), then f16 proj.
  - Output f16; host upcasts, sums partials, adds b_proj.

v3 scheduling: exp on ACT is the scarce serial resource (~73us busy) and
in v2 it idled ~23us in the first 40us then gated everything late.  v3
runs S/exp production as a single GLOBAL stream in consumption order,
emitted as early as data deps allow and paced against emitted-PE-work so
the s2 psum double-buffer never parks PE.  The attention rounds consume
pre-produced pt tiles (deep p_pt pool) and the qkv/v waves + projections
are finer-grained filler pieces metered between exp emissions.  The tail
projs use dedicated psum halves, parallel DVE+ACT staging copies, and
per-half DMAs so the last row-block's output leaves as early as possible.
"""

import numpy as np

B, T, C = 2, 2048, 1024
H, HD = 16, 64
NCORES = 8
HEADS_PER_CORE = 4          # 2 pairs
CH = HEADS_PER_CORE * HD    # 256 channels per core
KT8 = 4                     # fp8 contraction pair-tiles (K=256 each)
NT = T // 128               # 16 t tiles
NJ = T // 512               # 4 query chunks
SCALE = 1.0 / np.sqrt(HD)
WS = 64.0                   # host-side weight pre-scale for fp8 range

# scheduler tunables
PT_BUFS = 30                # pt pool depth (exp run-ahead)
PT_CAP = PT_BUFS - 4        # max exps in flight (emitted, not av-consumed)
MARGIN = 200.0              # ns of allowed projected PE-wait at an S matmul
WARM_INIT = 4               # big warm spins before the ramp matmuls
WARM_KK = 2                 # warm spins per kk ramp step

_COMPILED = None


def _build():
    import concourse.bass as bass
    import concourse.bacc as bacc
    import concourse.mybir as mybir
    import concourse.tile as tile

    f32 = mybir.dt.float32
    f16 = mybir.dt.float16
    f8 = mybir.dt.float8e4
    DR = mybir.MatmulPerfMode.DoubleRow
    Exp = mybir.ActivationFunctionType.Exp
    Ident = mybir.ActivationFunctionType.Identity
    mult = mybir.AluOpType.mult
    add = mybir.AluOpType.add

    nc = bacc.Bacc("TRN2", target_bir_lowering=False, debug=False)

    # combined main||residual fp8 tensors: one DMA feeds both comp8 terms
    x8_d = nc.dram_tensor("x8c", [KT8, 128, 2, 2, T], f8, kind="ExternalInput").ap()
    w8_d = nc.dram_tensor("w8c", [KT8, 128, 2, 2 * 3 * CH], f8, kind="ExternalInput").ap()
    wp_d = nc.dram_tensor("wp", [2, 128, C], f16, kind="ExternalInput").ap()
    bqk_d = nc.dram_tensor("bqk", [128, 4], f32, kind="ExternalInput").ap()
    bv_d = nc.dram_tensor("bv", [128, CH], f32, kind="ExternalInput").ap()
    tri_d = nc.dram_tensor("tri", [128, 128], f16, kind="ExternalInput").ap()
    ident_d = nc.dram_tensor("ident", [128, 128], f16, kind="ExternalInput").ap()
    out_d = nc.dram_tensor("out_p", [T, C], f16, kind="ExternalOutput").ap()

    with tile.TileContext(nc) as tc:
        with (
            tc.tile_pool(name="p_w", bufs=1) as p_w,
            tc.tile_pool(name="p_x", bufs=1) as p_x,
            tc.tile_pool(name="p_qk", bufs=1) as p_qk,
            tc.tile_pool(name="p_v", bufs=1) as p_v,
            tc.tile_pool(name="p_y", bufs=1) as p_y,
            tc.tile_pool(name="p_pt", bufs=PT_BUFS) as p_pt,
            tc.tile_pool(name="p_yn", bufs=16) as p_yn,
            tc.tile_pool(name="p_st", bufs=4) as p_st,
            tc.tile_pool(name="ps_s", bufs=2, space="PSUM") as ps_s,
            tc.tile_pool(name="ps_y", bufs=1, space="PSUM") as ps_y,
            tc.tile_pool(name="ps_mm", bufs=2, space="PSUM") as ps_mm,
        ):
            # ---- persistent inputs -------------------------------------
            ident = p_w.tile([128, 128], f16, name="ident", tag="ident")
            tri = p_w.tile([128, 128], f16, name="tri", tag="tri")
            bqk = p_w.tile([128, 4], f32, name="bqk", tag="bqk")
            bvb = p_w.tile([128, CH], f32, name="bvb", tag="bvb")
            w8c = [p_w.tile([128, 2, 2 * 3 * CH], f8, name=f"w8_{k}", tag=f"w8_{k}")
                   for k in range(KT8)]
            x8c = [p_x.tile([128, 2, 2, T], f8, name=f"x8_{k}", tag=f"x8_{k}")
                   for k in range(KT8)]
            wp = [p_w.tile([128, C], f16, name=f"wp{k}", tag=f"wp{k}")
                  for k in range(2)]
            # w8c columns: [pair0: q0m k0m q0r k0r | pair1: q1m k1m q1r k1r
            #               | v-main | v-resid] (128 cols each block)
            QKOFF = {0: (0, 256), 2: (128, 384), 1: (512, 768), 3: (640, 896)}

            def w_qk(kk, mi, g):
                moff, roff = QKOFF[mi]
                off = roff if g == 1 else moff
                return w8c[kk][:, :, off:off + 128]
            w8v = [w8c[k][:, :, 1024:1280] for k in range(KT8)]
            wr8v = [w8c[k][:, :, 1280:1536] for k in range(KT8)]
            x8 = [x8c[k][:, :, 0, :] for k in range(KT8)]
            xr8 = [x8c[k][:, :, 1, :] for k in range(KT8)]

            # warmup sources first (DVE memset, no deps) so PE can spin
            wsrc = p_w.tile([128, 128], f16, name="wsrc", tag="wsrc")
            nc.vector.memset(wsrc, 0.5)
            wsrc2 = p_w.tile([128, 512], f16, name="wsrc2", tag="wsrc2")
            nc.vector.memset(wsrc2, 0.5)
            # all transfers share one ~360B/ns pipe; order the bytes so only
            # the ramp-critical set [x-q1, w-pair0] leads.  SP carries the
            # ordered critical stream + pair1 + the x tail; Pool (SWDGE)
            # carries pair0 + late consts + v weights (issue-side overlap).
            nc.sync.dma_start(out=bqk, in_=bqk_d)
            for k in range(KT8):
                nc.sync.dma_start(out=x8c[k][:, :, :, 0:512],
                                  in_=x8_d[k][:, :, :, 0:512])
            for k in range(KT8):
                nc.sync.dma_start(out=w8c[k][:, :, 512:1024],
                                  in_=w8_d[k][:, :, 512:1024])
            for k in range(KT8):
                nc.gpsimd.dma_start(out=w8c[k][:, :, 0:512],
                                    in_=w8_d[k][:, :, 0:512])
            nc.gpsimd.dma_start(out=tri, in_=tri_d)
            nc.gpsimd.dma_start(out=bvb, in_=bv_d)
            for k in range(KT8):
                nc.gpsimd.dma_start(out=w8c[k][:, :, 1024:1536],
                                    in_=w8_d[k][:, :, 1024:1536])
            nc.gpsimd.dma_start(out=ident, in_=ident_d)
            for q in (1, 2, 3):
                for k in range(KT8):
                    nc.sync.dma_start(
                        out=x8c[k][:, :, :, 512 * q:512 * (q + 1)],
                        in_=x8_d[k][:, :, :, 512 * q:512 * (q + 1)])
            for k in range(2):
                nc.sync.dma_start(out=wp[k], in_=wp_d[k])

            # ---- persistent intermediates ------------------------------
            # qT/kT tile p: head pair p, heads (2p, 2p+1) on partitions 0:64/64:128
            qT = [p_qk.tile([128, T], f16, name=f"qT{p}", tag=f"qT{p}") for p in range(2)]
            kT = [p_qk.tile([128, T], f16, name=f"kT{p}", tag=f"kT{p}") for p in range(2)]
            # v tiles: [128 t, 4 heads, 65] -- col 0 of each head = 1.0 (denominator)
            v = [p_v.tile([128, 4, 65], f16, name=f"v{m}", tag=f"v{m}") for m in range(NT)]
            # normalized y^T per pair: [128 ch, T]
            ynT = [p_y.tile([128, T], f16, name=f"ynT{p}", tag=f"ynT{p}") for p in range(2)]

            # ---- global exp-stream scheduler state ----------------------
            # (3,1,0..11) jumps ahead of (3,0) so only the 4 diagonal chunks
            # of the final round trail on ACT at the very end.
            exp_tasks = []
            for j_ in range(NJ - 1):
                for p_ in range(2):
                    exp_tasks.extend((j_, p_, i_) for i_ in range(4 * j_ + 4))
            exp_tasks.extend((3, 1, i_) for i_ in range(12))
            exp_tasks.extend((3, 0, i_) for i_ in range(16))
            exp_tasks.extend((3, 1, i_) for i_ in range(12, 16))
            exp_pos = [0]
            pt_inflight = [0]
            qdone = set()           # (p, j) q chunks with the move emitted
            kcols = {0: 0, 1: 0}    # emitted kT column extent per pair
            prerolled = {}          # (j, p) -> {i: pt tile}
            # virtual clocks: pe_clock = cumulative emitted PE ns; act_fin[n]
            # = projected finish of exp n.  The s2 psum double-buffer means
            # S(n+2) parks PE until exp(n) completes, so exp n is emitted
            # paced only once pe_clock has caught up to act_fin[n-2].
            pe_clock = [0.0]
            act_fin = []

            def pe_work(ns):
                pe_clock[0] += ns

            # PE p-state warmup: cheap dependency-light matmuls. Target lives
            # in the y-pool, which nothing touches until attention(0,0)'s
            # first AV (well after the ramp).
            warm = ps_y.tile([128, 512], f32, name="warm", tag="y0")

            def warm_mms(n):
                for _ in range(n):
                    nc.tensor.matmul(warm, lhsT=wsrc, rhs=wsrc2,
                                     start=True, stop=True)
                pe_work(n * 213.0)
            # first spins only need the small (fast) memset source
            for _ in range(4):
                nc.tensor.matmul(warm[:, 0:128], lhsT=wsrc, rhs=wsrc,
                                 start=True, stop=True)
            warm_mms(WARM_INIT)

            def qkv_mms(ps, mi, nj, glist):
                for g in glist:
                    xa = xr8 if g == 2 else x8
                    for kk in range(KT8):
                        nc.tensor.matmul(
                            ps[:, 0:512],
                            lhsT=w_qk(kk, mi, g),
                            rhs=xa[kk][:, :, 512 * nj:512 * (nj + 1)],
                            start=(g == 0 and kk == 0),
                            stop=(g == 2 and kk == KT8 - 1),
                            perf_mode=DR,
                        )
                pe_work(len(glist) * 4 * 107.0)

            def qkv_move(ps, mi, nj):
                dst = (qT if mi < 2 else kT)[mi % 2][:, 512 * nj:512 * (nj + 1)]
                nc.vector.tensor_scalar(dst, ps[:, 0:512], 1.0 / WS,
                                        bqk[:, mi:mi + 1], mult, add)
                if mi < 2:
                    qdone.add((mi, nj))
                else:
                    kcols[mi % 2] = max(kcols[mi % 2], 512 * (nj + 1))

            def qkv_chunk(mi, nj):
                """q/k channels [128mi,128mi+128), t [512nj, 512nj+512)."""
                ps = ps_mm.tile([128, 512], f32, name="ps_qkv", tag="mm")
                qkv_mms(ps, mi, nj, (0, 1, 2))
                qkv_move(ps, mi, nj)

            def qkv_pieces(mi, nj):
                """qkv chunk as 2 filler pieces (~0.85/0.43us)."""
                st = {}

                def p0():
                    st["ps"] = ps_mm.tile([128, 512], f32, name="ps_qkv",
                                          tag="mm")
                    qkv_mms(st["ps"], mi, nj, (0, 1))
                return [p0,
                        lambda: (qkv_mms(st["ps"], mi, nj, (2,)),
                                 qkv_move(st["ps"], mi, nj))]

            def v_mms(ps, m, glist):
                for g in glist:
                    xa = xr8 if g == 2 else x8
                    wb = wr8v if g == 1 else w8v
                    for kk in range(KT8):
                        nc.tensor.matmul(
                            ps[:, 0:CH],
                            lhsT=xa[kk][:, :, 128 * m:128 * (m + 1)],
                            rhs=wb[kk],
                            start=(g == 0 and kk == 0),
                            stop=(g == 2 and kk == KT8 - 1),
                            perf_mode=DR,
                        )
                pe_work(len(glist) * 4 * 53.0)

            def v_move(ps, m):
                nc.vector.memset(v[m][:, :, 0:1], 1.0)
                nc.vector.scalar_tensor_tensor(
                    v[m][:, :, 1:65],
                    ps[:, 0:CH].rearrange("p (h c) -> p h c", h=4),
                    1.0 / WS,
                    bvb.rearrange("p (h c) -> p h c", h=4),
                    mult, add,
                )

            def v_pieces(m):
                st = {}

                def p0():
                    st["ps"] = ps_mm.tile([128, 512], f32, name="ps_v",
                                          tag="mm")
                    v_mms(st["ps"], m, (0, 1))
                return [p0,
                        lambda: (v_mms(st["ps"], m, (2,)), v_move(st["ps"], m))]

            def v_chunk(m):
                ps = ps_mm.tile([128, 512], f32, name="ps_v", tag="mm")
                v_mms(ps, m, (0, 1, 2))
                v_move(ps, m)

            # ---- S/exp production --------------------------------------
            def s_exp_chunk(j, p, i):
                """S matmuls + fused exp (+ diag tri-mask) for one chunk."""
                rr = i - 4 * j
                W0 = 128 * rr if rr > 0 else 0
                s2 = ps_s.tile([128, 2, 512], f32, name="s2", tag="s")
                for h in range(2):
                    nc.tensor.matmul(
                        s2[:, h, W0:512],
                        lhsT=kT[p][64 * h:64 * h + 64, 128 * i:128 * (i + 1)],
                        rhs=qT[p][64 * h:64 * h + 64, 512 * j + W0:512 * (j + 1)],
                        start=True, stop=True,
                    )
                pt = p_pt.tile([128, 2, 512], f16, name="pt", tag="pt")
                nc.scalar.activation(pt[:, :, W0:512], s2[:, :, W0:512], Exp)
                if rr >= 0:
                    for h in range(2):
                        nc.gpsimd.tensor_tensor(
                            pt[:, h, W0:W0 + 128], pt[:, h, W0:W0 + 128],
                            tri, mult)
                return pt

            def exp_ready(t):
                j, p, i = t
                return (p, j) in qdone and kcols[p] >= 128 * (i + 1)

            def emit_exp():
                j, p, i = exp_tasks[exp_pos[0]]
                exp_pos[0] += 1
                pt = s_exp_chunk(j, p, i)
                prerolled.setdefault((j, p), {})[i] = pt
                pt_inflight[0] += 1
                rr = i - 4 * j
                cols = 2 * (512 - (128 * rr if rr > 0 else 0))
                pe_work(cols * 0.4167)
                start = max(act_fin[-1] if act_fin else 0.0, pe_clock[0])
                act_fin.append(start + cols * 0.8333 + 185.0)

            def maybe_emit_exp(force=None):
                """Emit paced exps from the global stream.  With force=(j,p,i)
                emit stream entries until that task is out, regardless of
                pacing (the consumer needs it now)."""
                while exp_pos[0] < len(exp_tasks):
                    t = exp_tasks[exp_pos[0]]
                    if force is not None:
                        fj, fp, fi = force
                        if fi in prerolled.get((fj, fp), {}):
                            force = None
                            continue
                        assert exp_ready(t), ("forced emit with unmet dep", t, force)
                        emit_exp()
                        continue
                    if not exp_ready(t) or pt_inflight[0] >= PT_CAP:
                        break
                    n = len(act_fin)
                    if n >= 2 and pe_clock[0] < act_fin[n - 2] - MARGIN:
                        break
                    emit_exp()

            # ---- attention rounds (pt consumers) ------------------------
            def attention(j, p, filler, depth=2, tail_hook=None, fps=1):
                """q-chunk j (512 queries), head pair p (heads 2p, 2p+1)."""
                ni = 4 * j + 4
                yp = [ps_y.tile([128, 4, 65], f32, name=f"y{h}", tag=f"y{h}")
                      for h in range(2)]
                pts = prerolled.setdefault((j, p), {})

                deferred = []

                def av(i):
                    rr = i - 4 * j
                    pt = pts.pop(i)
                    pt_inflight[0] -= 1
                    ntt = 4 - max(0, rr)
                    for h in range(2):
                        for tt in range(max(0, rr), 4):
                            nc.tensor.matmul(
                                yp[h][:, tt, 0:65],
                                lhsT=pt[:, h, 128 * tt:128 * (tt + 1)],
                                rhs=v[i][:, 2 * p + h, :],
                                start=(i == 0 and tt == 0),
                                stop=(i == 4 * j + tt),
                                skip_group_check=True,
                            )
                    pe_work(2 * ntt * 27.0)
                    # q-subtile tt's accumulation closed at chunk 4j+tt:
                    # stream its normalize (+ tail work) immediately
                    tt = i - 4 * j
                    if tt >= 0:
                        norm(tt)

                def norm(tt):
                    # ISA TensorScalar has no divide: per-head reciprocal of
                    # the denominator (col 0), then scalar-multiply.  In the
                    # tail the two heads' scales run on DVE and Pool in
                    # parallel (the chain is latency-critical there).
                    rc = p_yn.tile([128, 2], f32, name="rc", tag="rc", bufs=8)
                    ynst = p_yn.tile([128, 128], f16, name="ynst", tag="yn")
                    for h in range(2):
                        nc.vector.reciprocal(rc[:, h:h + 1], yp[h][:, tt, 0:1])
                        nc.vector.tensor_scalar(
                            ynst[:, 64 * h:64 * h + 64],
                            yp[h][:, tt, 1:65], rc[:, h:h + 1], None, mult)

                    def transpose_move(tt=tt, ynst=ynst):
                        tp = ps_mm.tile([128, 128], f16, name="tp", tag="mm")
                        nc.tensor.transpose(tp, ynst, ident)
                        nc.vector.tensor_copy(
                            ynT[p][:, 512 * j + 128 * tt:
                                   512 * j + 128 * (tt + 1)], tp)
                        pe_work(53.0)
                    if tail_hook is not None:
                        transpose_move()
                        tail_hook(tt)
                    else:
                        deferred.append(transpose_move)

                for i in range(ni):
                    if i not in pts:
                        maybe_emit_exp(force=(j, p, i))
                        assert i in pts
                    if i >= depth:
                        av(i - depth)
                    for _ in range(fps):
                        maybe_emit_exp()
                        filler()
                    maybe_emit_exp()
                for i in range(ni - depth, ni):
                    av(i)
                    maybe_emit_exp()
                    if tail_hook is None:
                        filler()
                        maybe_emit_exp()
                prerolled.pop((j, p), None)
                return deferred

            # ---- projection --------------------------------------------
            def proj_pieces(m):
                """output rows [128m, 128m+128) as 2 filler pieces; staging
                copies on DVE, full-row DMA via SWDGE at the end."""
                st = {"st": None}

                def pu(u):
                    if u == 0:
                        st["st"] = p_st.tile([128, 1024], f16, name="st_pr",
                                             tag="st")
                    ps = ps_mm.tile([128, 512], f32, name="ps_pr", tag="mm")
                    for kk in range(2):
                        nc.tensor.matmul(
                            ps[:, 0:512],
                            lhsT=ynT[kk][:, 128 * m:128 * (m + 1)],
                            rhs=wp[kk][:, 512 * u:512 * (u + 1)],
                            start=(kk == 0), stop=(kk == 1),
                        )
                    pe_work(2 * 213.0)
                    if exp_pos[0] >= len(exp_tasks):
                        nc.scalar.copy(
                            st["st"][:, 512 * u:512 * (u + 1)], ps[:, 0:512])
                    else:
                        nc.vector.tensor_copy(
                            st["st"][:, 512 * u:512 * (u + 1)], ps[:, 0:512])
                    if u == 1:
                        nc.sync.dma_start(
                            out=out_d[128 * m:128 * (m + 1), :], in_=st["st"])
                return [lambda: pu(0), lambda: pu(1)]

            def proj_tail(m, tt):
                """tail proj: dedicated ps_s halves, DVE+ACT copies in
                parallel, per-half DMAs fired as each copy lands."""
                st = p_st.tile([128, 1024], f16, name="st_pr", tag="st")
                ps = ps_s.tile([128, 2, 512], f32, name="ps_tl", tag="s")
                for u in range(2):
                    for kk in range(2):
                        nc.tensor.matmul(
                            ps[:, u, 0:512],
                            lhsT=ynT[kk][:, 128 * m:128 * (m + 1)],
                            rhs=wp[kk][:, 512 * u:512 * (u + 1)],
                            start=(kk == 0), stop=(kk == 1),
                        )
                pe_work(4 * 213.0)
                nc.scalar.copy(st[:, 0:512], ps[:, 0, 0:512])
                nc.sync.dma_start(out=out_d[128 * m:128 * (m + 1), 0:512],
                                  in_=st[:, 0:512])
                nc.vector.tensor_copy(st[:, 512:1024], ps[:, 1, 0:512])
                nc.sync.dma_start(out=out_d[128 * m:128 * (m + 1), 512:1024],
                                  in_=st[:, 512:1024])

            # ---- filler machinery ---------------------------------------
            waves = []
            ordered = []

            def filler():
                if waves:
                    waves.pop(0)()
                elif ordered:
                    ordered.pop(0)()

            def drain(lst):
                while lst:
                    lst.pop(0)()

            # ---- emission order (scheduling priority) -------------------
            # ramp: kk-major over the [x8q1[k], w8qk[k]] DMA arrival stream.
            # ps_a accumulates the full q(0,0) chunk and ps_b the full k(2,0)
            # chunk; the ~1.28us of real matmul work per kk matches the
            # ~1.27us DMA cadence, so no warm filler is needed inside the
            # loop and the first S/exp can fire the moment the moves land.
            ps_a = ps_mm.tile([128, 512], f32, name="ps_qkv", tag="mm")
            ps_b = ps_mm.tile([128, 512], f32, name="ps_qkv", tag="mm")
            for kk in range(KT8):
                for ps, mi in ((ps_b, 2), (ps_a, 0)):
                    for g in range(3):
                        xa = xr8 if g == 2 else x8
                        nc.tensor.matmul(
                            ps[:, 0:512],
                            lhsT=w_qk(kk, mi, g),
                            rhs=xa[kk][:, :, 0:512],
                            start=(kk == 0 and g == 0),
                            stop=(kk == KT8 - 1 and g == 2),
                            perf_mode=DR,
                        )
                pe_work(6 * 213.0)
            qkv_move(ps_b, 2, 0)
            # q-move on ACT (Identity w/ scale+bias) in parallel with the
            # DVE k-move: the first S needs both
            nc.scalar.activation(qT[0][:, 0:512], ps_a[:, 0:512], Ident,
                                 bias=bqk[:, 0:1], scale=1.0 / WS)
            qdone.add((0, 0))
            # (1,0) pieces keep PE dense while the DVE moves land; the first
            # S/exp chunks slot in between.
            pc = qkv_pieces(1, 0)
            pc[0]()
            maybe_emit_exp()
            pc[1]()
            maybe_emit_exp()
            pc = qkv_pieces(3, 0)
            pc[0]()
            maybe_emit_exp()
            pc[1]()
            maybe_emit_exp()

            # PH0: pace the remaining (0,*) exps against v(0..3) + the j=1
            # qkv waves.  Pair-0 chunks first: they unlock (1,0,*) exps.
            ph0 = []
            for mi in (0, 2, 1, 3):
                ph0.extend(qkv_pieces(mi, 1))
            for m_ in range(4):
                ph0.extend(v_pieces(m_))
            for piece in ph0:
                piece()
                maybe_emit_exp()

            # waves for round j carry the j+2 qkv chunks (the global exp
            # stream runs ~2 rounds ahead) and the j+1 v chunks.
            # ordered: transposes + projs -- span boundaries.
            for j in range(NJ):
                if j < 2:
                    for mi in (0, 2, 1, 3):
                        waves.extend(qkv_pieces(mi, j + 2))
                for m_ in range(4 * j + 4, 4 * j + 8):
                    if m_ < NT:
                        waves.extend(v_pieces(m_))
                if j == 2:
                    for m_ in range(0, 4):
                        ordered.extend(proj_pieces(m_))
                elif j == 3:
                    for m_ in range(4, 12):
                        ordered.extend(proj_pieces(m_))
                for p in range(2):
                    if (j, p) == (3, 1):
                        def tail_hook(tt):
                            drain(ordered)
                            proj_tail(12 + tt, tt)
                        attention(j, p, filler, tail_hook=tail_hook)
                    else:
                        deferred = attention(j, p, filler, depth=2,
                                             fps=(2 if (j, p) == (3, 0) else 1))
                        ordered.extend(deferred)
                drain(waves)

    nc.compile()
    return nc


def _host_inputs(x, W_attn, b_attn, W_proj):
    """Build the 8 per-core input maps (numpy only)."""
    import ml_dtypes
    f8 = ml_dtypes.float8_e4m3

    x = np.asarray(x, dtype=np.float32)
    W_attn = np.asarray(W_attn, dtype=np.float32)
    b_attn = np.asarray(b_attn, dtype=np.float32)
    W_proj = np.asarray(W_proj, dtype=np.float32)

    # strict causal 0/1 mask for the 128x128 diagonal blocks: valid iff c >= k
    kl = np.arange(128)
    tri = (kl[None, :] >= kl[:, None]).astype(np.float16)
    ident = np.eye(128, dtype=np.float16)

    def pack8(a):
        """[C, N] -> fp8 main/residual tiles [KT8, 128, 2, N] each."""
        a8 = a.astype(f8)
        ar8 = (a - a8.astype(np.float32)).astype(f8)
        def t(z):
            return z.reshape(KT8, 2, 128, a.shape[1]).transpose(0, 2, 1, 3)
        return t(a8), t(ar8)

    in_maps = []
    for c in range(NCORES):
        b, g = divmod(c, 4)
        sl = slice(CH * g, CH * (g + 1))
        wq = W_attn[:, 0 * C:1 * C][:, sl] * SCALE
        wk = W_attn[:, 1 * C:2 * C][:, sl]
        wv = W_attn[:, 2 * C:3 * C][:, sl]
        bq = b_attn[0 * C:1 * C][sl] * SCALE
        bk = b_attn[1 * C:2 * C][sl]
        bv = b_attn[2 * C:3 * C][sl]
        bqk = np.stack([bq[0:128], bq[128:256], bk[0:128], bk[128:256]], axis=1)
        wfull = np.concatenate([wq, wk, wv], axis=1) * WS     # [1024, 768]
        w8, wr8 = pack8(wfull)
        xT = np.ascontiguousarray(x[b].T)                     # [1024, 2048]
        x8, xr8 = pack8(xT)
        # columns: [p0: q0m k0m q0r k0r | p1: q1m k1m q1r k1r | vm | vr]
        w8c = np.ascontiguousarray(np.concatenate(
            [w8[..., 0:128], w8[..., 256:384],
             wr8[..., 0:128], wr8[..., 256:384],
             w8[..., 128:256], w8[..., 384:512],
             wr8[..., 128:256], wr8[..., 384:512],
             w8[..., 512:768], wr8[..., 512:768]], axis=3))
        x8c = np.ascontiguousarray(np.stack([x8, xr8], axis=3))
        in_maps.append({
            "x8c": x8c, "w8c": w8c,
            "wp": np.ascontiguousarray(
                W_proj[sl, :].reshape(2, 128, C).astype(np.float16)),
            "bqk": np.ascontiguousarray(bqk),
            "bv": np.ascontiguousarray(
                np.broadcast_to(bv[None, :], (128, CH))),
            "tri": tri, "ident": ident,
        })
    return in_maps


def kernel(x, W_attn, b_attn, W_proj, b_proj, _want_results=None):
    global _COMPILED
    from concourse.bass_utils import run_bass_kernel_spmd

    if _COMPILED is None:
        _COMPILED = _build()
    nc = _COMPILED

    in_maps = _host_inputs(x, W_attn, b_attn, W_proj)
    kw = dict(_want_results or {})
    res = run_bass_kernel_spmd(nc, in_maps, core_ids=list(range(NCORES)), **kw)
    if _want_results is not None:
        kernel.last_results = res

    out = np.zeros((B, T, C), dtype=np.float32)
    for c in range(NCORES):
        out[c // 4] += res.results[c]["out_p"].astype(np.float32)
    out += np.asarray(b_proj, dtype=np.float32)[None, None, :]
    return out


# revision 51
# speedup vs baseline: 1.0282x; 1.0124x over previous
"""Causal self-attention (B=2, T=2048, C=1024, H=16) on 8 TRN2 NeuronCores.

Sharding: core c -> batch b = c//4, head-group g = c%4 (4 heads = 256 channels).
Each core computes its 4 heads end-to-end and a partial projection
(y_local @ W_proj[256g:256g+256, :]); the host sums the 4 partials per batch.

v3 dataflow (cost-model-driven; v2 was 111450 ns):
  - QKV/V matmuls in fp8e4m3 DoubleRow with two-term compensation:
    (x8+xr8)@(w8+wr8) dropping xr8@wr8 -> 12 DR passes per chunk at
    0.5 cyc/col, weights pre-scaled x64 on host (descaled in the
    psum->sbuf move) to clear fp8's subnormal floor.
  - S^T = k_h^T q_h per head in f16, exact 128-granular causal windows.
    Both heads of a pair share one [128,2,512] psum tile; one fused exp
    per chunk on ACT.  Causal masking via 0/1 f16 multiply on DVE after
    exp.
  - AV in natural [q,d] layout: lhsT = pt q-slices, rhs = [1|v_h] ->
    65-col matmuls; softmax denominator rides column 0; normalize is a
    reciprocal + per-partition tensor_scalar multiply.
  - Normalized y transposed back via PE is_transpose (f16), then f16 proj.
  - Output f16; host upcasts, sums partials, adds b_proj.

v3 scheduling: exp on ACT is the scarce serial resource (~73us busy) and
in v2 it idled ~23us in the first 40us then gated everything late.  v3
runs S/exp production as a single GLOBAL stream in consumption order,
emitted as early as data deps allow and paced against emitted-PE-work so
the s2 psum double-buffer never parks PE.  The attention rounds consume
pre-produced pt tiles (deep p_pt pool) and the qkv/v waves + projections
are finer-grained filler pieces metered between exp emissions.  The tail
projs use dedicated psum halves, parallel DVE+ACT staging copies, and
per-half DMAs so the last row-block's output leaves as early as possible.
"""

import numpy as np

B, T, C = 2, 2048, 1024
H, HD = 16, 64
NCORES = 8
HEADS_PER_CORE = 4          # 2 pairs
CH = HEADS_PER_CORE * HD    # 256 channels per core
KT8 = 4                     # fp8 contraction pair-tiles (K=256 each)
NT = T // 128               # 16 t tiles
NJ = T // 512               # 4 query chunks
SCALE = 1.0 / np.sqrt(HD)
WS = 64.0                   # host-side weight pre-scale for fp8 range

# scheduler tunables
PT_BUFS = 30                # pt pool depth (exp run-ahead)
PT_CAP = PT_BUFS - 4        # max exps in flight (emitted, not av-consumed)
MARGIN = 800.0              # ns of allowed projected PE-wait at an S matmul
WARM_INIT = 4               # big warm spins before the ramp matmuls
WARM_KK = 2                 # warm spins per kk ramp step

_COMPILED = None


def _build():
    import concourse.bass as bass
    import concourse.bacc as bacc
    import concourse.mybir as mybir
    import concourse.tile as tile

    f32 = mybir.dt.float32
    f16 = mybir.dt.float16
    f8 = mybir.dt.float8e4
    DR = mybir.MatmulPerfMode.DoubleRow
    Exp = mybir.ActivationFunctionType.Exp
    Ident = mybir.ActivationFunctionType.Identity
    mult = mybir.AluOpType.mult
    add = mybir.AluOpType.add

    nc = bacc.Bacc("TRN2", target_bir_lowering=False, debug=False)

    # combined main||residual fp8 tensors: one DMA feeds both comp8 terms
    x8_d = nc.dram_tensor("x8c", [KT8, 128, 2, 2, T], f8, kind="ExternalInput").ap()
    w8_d = nc.dram_tensor("w8c", [KT8, 128, 2, 2 * 3 * CH], f8, kind="ExternalInput").ap()
    wp_d = nc.dram_tensor("wp", [2, 128, C], f16, kind="ExternalInput").ap()
    bqk_d = nc.dram_tensor("bqk", [128, 4], f32, kind="ExternalInput").ap()
    bv_d = nc.dram_tensor("bv", [128, CH], f32, kind="ExternalInput").ap()
    tri_d = nc.dram_tensor("tri", [128, 128], f16, kind="ExternalInput").ap()
    ident_d = nc.dram_tensor("ident", [128, 128], f16, kind="ExternalInput").ap()
    out_d = nc.dram_tensor("out_p", [T, C], f16, kind="ExternalOutput").ap()

    with tile.TileContext(nc) as tc:
        with (
            tc.tile_pool(name="p_w", bufs=1) as p_w,
            tc.tile_pool(name="p_x", bufs=1) as p_x,
            tc.tile_pool(name="p_qk", bufs=1) as p_qk,
            tc.tile_pool(name="p_v", bufs=1) as p_v,
            tc.tile_pool(name="p_y", bufs=1) as p_y,
            tc.tile_pool(name="p_pt", bufs=PT_BUFS) as p_pt,
            tc.tile_pool(name="p_yn", bufs=16) as p_yn,
            tc.tile_pool(name="p_st", bufs=4) as p_st,
            tc.tile_pool(name="ps_s", bufs=2, space="PSUM") as ps_s,
            tc.tile_pool(name="ps_y", bufs=1, space="PSUM") as ps_y,
            tc.tile_pool(name="ps_mm", bufs=2, space="PSUM") as ps_mm,
        ):
            # ---- persistent inputs -------------------------------------
            ident = p_w.tile([128, 128], f16, name="ident", tag="ident")
            tri = p_w.tile([128, 128], f16, name="tri", tag="tri")
            bqk = p_w.tile([128, 4], f32, name="bqk", tag="bqk")
            bvb = p_w.tile([128, CH], f32, name="bvb", tag="bvb")
            w8c = [p_w.tile([128, 2, 2 * 3 * CH], f8, name=f"w8_{k}", tag=f"w8_{k}")
                   for k in range(KT8)]
            x8c = [p_x.tile([128, 2, 2, T], f8, name=f"x8_{k}", tag=f"x8_{k}")
                   for k in range(KT8)]
            wp = [p_w.tile([128, C], f16, name=f"wp{k}", tag=f"wp{k}")
                  for k in range(2)]
            # w8c columns: [pair0: q0m k0m q0r k0r | pair1: q1m k1m q1r k1r
            #               | v-main | v-resid] (128 cols each block)
            QKOFF = {0: (0, 256), 2: (128, 384), 1: (512, 768), 3: (640, 896)}

            def w_qk(kk, mi, g):
                moff, roff = QKOFF[mi]
                off = roff if g == 1 else moff
                return w8c[kk][:, :, off:off + 128]
            w8v = [w8c[k][:, :, 1024:1280] for k in range(KT8)]
            wr8v = [w8c[k][:, :, 1280:1536] for k in range(KT8)]
            x8 = [x8c[k][:, :, 0, :] for k in range(KT8)]
            xr8 = [x8c[k][:, :, 1, :] for k in range(KT8)]

            # warmup sources first (DVE memset, no deps) so PE can spin
            wsrc = p_w.tile([128, 128], f16, name="wsrc", tag="wsrc")
            nc.vector.memset(wsrc, 0.5)
            wsrc2 = p_w.tile([128, 512], f16, name="wsrc2", tag="wsrc2")
            nc.vector.memset(wsrc2, 0.5)
            # all transfers share one ~360B/ns pipe; order the bytes so only
            # the ramp-critical set [x-q1, w-pair0] leads.  SP carries the
            # ordered critical stream + pair1 + the x tail; Pool (SWDGE)
            # carries pair0 + late consts + v weights (issue-side overlap).
            nc.sync.dma_start(out=bqk, in_=bqk_d)
            for k in range(KT8):
                nc.sync.dma_start(out=x8c[k][:, :, :, 0:512],
                                  in_=x8_d[k][:, :, :, 0:512])
            for k in range(KT8):
                nc.sync.dma_start(out=w8c[k][:, :, 512:1024],
                                  in_=w8_d[k][:, :, 512:1024])
            for k in range(KT8):
                nc.gpsimd.dma_start(out=w8c[k][:, :, 0:512],
                                    in_=w8_d[k][:, :, 0:512])
            nc.gpsimd.dma_start(out=tri, in_=tri_d)
            nc.gpsimd.dma_start(out=bvb, in_=bv_d)
            for k in range(KT8):
                nc.gpsimd.dma_start(out=w8c[k][:, :, 1024:1536],
                                    in_=w8_d[k][:, :, 1024:1536])
            nc.gpsimd.dma_start(out=ident, in_=ident_d)
            for q in (1, 2, 3):
                for k in range(KT8):
                    nc.sync.dma_start(
                        out=x8c[k][:, :, :, 512 * q:512 * (q + 1)],
                        in_=x8_d[k][:, :, :, 512 * q:512 * (q + 1)])
            for k in range(2):
                nc.sync.dma_start(out=wp[k], in_=wp_d[k])

            # ---- persistent intermediates ------------------------------
            # qT/kT tile p: head pair p, heads (2p, 2p+1) on partitions 0:64/64:128
            qT = [p_qk.tile([128, T], f16, name=f"qT{p}", tag=f"qT{p}") for p in range(2)]
            kT = [p_qk.tile([128, T], f16, name=f"kT{p}", tag=f"kT{p}") for p in range(2)]
            # v tiles: [128 t, 4 heads, 65] -- col 0 of each head = 1.0 (denominator)
            v = [p_v.tile([128, 4, 65], f16, name=f"v{m}", tag=f"v{m}") for m in range(NT)]
            # normalized y^T per pair: [128 ch, T]
            ynT = [p_y.tile([128, T], f16, name=f"ynT{p}", tag=f"ynT{p}") for p in range(2)]

            # ---- global exp-stream scheduler state ----------------------
            # (3,1,0..11) jumps ahead of (3,0) so only the 4 diagonal chunks
            # of the final round trail on ACT at the very end.
            exp_tasks = []
            for j_ in range(NJ - 1):
                for p_ in range(2):
                    exp_tasks.extend((j_, p_, i_) for i_ in range(4 * j_ + 4))
            exp_tasks.extend((3, 1, i_) for i_ in range(12))
            exp_tasks.extend((3, 0, i_) for i_ in range(16))
            exp_tasks.extend((3, 1, i_) for i_ in range(12, 16))
            exp_pos = [0]
            pt_inflight = [0]
            qdone = set()           # (p, j) q chunks with the move emitted
            kcols = {0: 0, 1: 0}    # emitted kT column extent per pair
            prerolled = {}          # (j, p) -> {i: pt tile}
            # virtual clocks: pe_clock = cumulative emitted PE ns; act_fin[n]
            # = projected finish of exp n.  The s2 psum double-buffer means
            # S(n+2) parks PE until exp(n) completes, so exp n is emitted
            # paced only once pe_clock has caught up to act_fin[n-2].
            pe_clock = [0.0]
            act_fin = []

            def pe_work(ns):
                pe_clock[0] += ns

            # PE p-state warmup: cheap dependency-light matmuls. Target lives
            # in the y-pool, which nothing touches until attention(0,0)'s
            # first AV (well after the ramp).
            warm = ps_y.tile([128, 512], f32, name="warm", tag="y0")

            def warm_mms(n):
                for _ in range(n):
                    nc.tensor.matmul(warm, lhsT=wsrc, rhs=wsrc2,
                                     start=True, stop=True)
                pe_work(n * 213.0)
            # first spins only need the small (fast) memset source
            for _ in range(4):
                nc.tensor.matmul(warm[:, 0:128], lhsT=wsrc, rhs=wsrc,
                                 start=True, stop=True)
            warm_mms(WARM_INIT)

            def qkv_mms(ps, mi, nj, glist):
                for g in glist:
                    xa = xr8 if g == 2 else x8
                    for kk in range(KT8):
                        nc.tensor.matmul(
                            ps[:, 0:512],
                            lhsT=w_qk(kk, mi, g),
                            rhs=xa[kk][:, :, 512 * nj:512 * (nj + 1)],
                            start=(g == 0 and kk == 0),
                            stop=(g == 2 and kk == KT8 - 1),
                            perf_mode=DR,
                        )
                pe_work(len(glist) * 4 * 107.0)

            def qkv_move(ps, mi, nj):
                dst = (qT if mi < 2 else kT)[mi % 2][:, 512 * nj:512 * (nj + 1)]
                nc.vector.tensor_scalar(dst, ps[:, 0:512], 1.0 / WS,
                                        bqk[:, mi:mi + 1], mult, add)
                if mi < 2:
                    qdone.add((mi, nj))
                else:
                    kcols[mi % 2] = max(kcols[mi % 2], 512 * (nj + 1))

            def qkv_chunk(mi, nj):
                """q/k channels [128mi,128mi+128), t [512nj, 512nj+512)."""
                ps = ps_mm.tile([128, 512], f32, name="ps_qkv", tag="mm")
                qkv_mms(ps, mi, nj, (0, 1, 2))
                qkv_move(ps, mi, nj)

            def qkv_pieces(mi, nj):
                """qkv chunk as 2 filler pieces (~0.85/0.43us)."""
                st = {}

                def p0():
                    st["ps"] = ps_mm.tile([128, 512], f32, name="ps_qkv",
                                          tag="mm")
                    qkv_mms(st["ps"], mi, nj, (0, 1))
                return [p0,
                        lambda: (qkv_mms(st["ps"], mi, nj, (2,)),
                                 qkv_move(st["ps"], mi, nj))]

            def v_mms(ps, m, glist):
                for g in glist:
                    xa = xr8 if g == 2 else x8
                    wb = wr8v if g == 1 else w8v
                    for kk in range(KT8):
                        nc.tensor.matmul(
                            ps[:, 0:CH],
                            lhsT=xa[kk][:, :, 128 * m:128 * (m + 1)],
                            rhs=wb[kk],
                            start=(g == 0 and kk == 0),
                            stop=(g == 2 and kk == KT8 - 1),
                            perf_mode=DR,
                        )
                pe_work(len(glist) * 4 * 53.0)

            def v_move(ps, m):
                nc.vector.memset(v[m][:, :, 0:1], 1.0)
                nc.vector.scalar_tensor_tensor(
                    v[m][:, :, 1:65],
                    ps[:, 0:CH].rearrange("p (h c) -> p h c", h=4),
                    1.0 / WS,
                    bvb.rearrange("p (h c) -> p h c", h=4),
                    mult, add,
                )

            def v_pieces(m):
                st = {}

                def p0():
                    st["ps"] = ps_mm.tile([128, 512], f32, name="ps_v",
                                          tag="mm")
                    v_mms(st["ps"], m, (0, 1))
                return [p0,
                        lambda: (v_mms(st["ps"], m, (2,)), v_move(st["ps"], m))]

            def v_chunk(m):
                ps = ps_mm.tile([128, 512], f32, name="ps_v", tag="mm")
                v_mms(ps, m, (0, 1, 2))
                v_move(ps, m)

            # ---- S/exp production --------------------------------------
            def s_exp_chunk(j, p, i):
                """S matmuls + fused exp (+ diag tri-mask) for one chunk."""
                rr = i - 4 * j
                W0 = 128 * rr if rr > 0 else 0
                s2 = ps_s.tile([128, 2, 512], f32, name="s2", tag="s")
                for h in range(2):
                    nc.tensor.matmul(
                        s2[:, h, W0:512],
                        lhsT=kT[p][64 * h:64 * h + 64, 128 * i:128 * (i + 1)],
                        rhs=qT[p][64 * h:64 * h + 64, 512 * j + W0:512 * (j + 1)],
                        start=True, stop=True,
                    )
                pt = p_pt.tile([128, 2, 512], f16, name="pt", tag="pt")
                nc.scalar.activation(pt[:, :, W0:512], s2[:, :, W0:512], Exp)
                if rr >= 0:
                    for h in range(2):
                        nc.gpsimd.tensor_tensor(
                            pt[:, h, W0:W0 + 128], pt[:, h, W0:W0 + 128],
                            tri, mult)
                return pt

            def exp_ready(t):
                j, p, i = t
                return (p, j) in qdone and kcols[p] >= 128 * (i + 1)

            def emit_exp():
                j, p, i = exp_tasks[exp_pos[0]]
                exp_pos[0] += 1
                pt = s_exp_chunk(j, p, i)
                prerolled.setdefault((j, p), {})[i] = pt
                pt_inflight[0] += 1
                rr = i - 4 * j
                cols = 2 * (512 - (128 * rr if rr > 0 else 0))
                pe_work(cols * 0.4167)
                start = max(act_fin[-1] if act_fin else 0.0, pe_clock[0])
                act_fin.append(start + cols * 0.8333 + 185.0)

            def maybe_emit_exp(force=None):
                """Emit paced exps from the global stream.  With force=(j,p,i)
                emit stream entries until that task is out, regardless of
                pacing (the consumer needs it now)."""
                while exp_pos[0] < len(exp_tasks):
                    t = exp_tasks[exp_pos[0]]
                    if force is not None:
                        fj, fp, fi = force
                        if fi in prerolled.get((fj, fp), {}):
                            force = None
                            continue
                        assert exp_ready(t), ("forced emit with unmet dep", t, force)
                        emit_exp()
                        continue
                    if not exp_ready(t) or pt_inflight[0] >= PT_CAP:
                        break
                    n = len(act_fin)
                    if n >= 2 and pe_clock[0] < act_fin[n - 2] - MARGIN:
                        break
                    emit_exp()

            # ---- attention rounds (pt consumers) ------------------------
            def attention(j, p, filler, depth=2, tail_hook=None, fps=1):
                """q-chunk j (512 queries), head pair p (heads 2p, 2p+1)."""
                ni = 4 * j + 4
                yp = [ps_y.tile([128, 4, 65], f32, name=f"y{h}", tag=f"y{h}")
                      for h in range(2)]
                pts = prerolled.setdefault((j, p), {})

                deferred = []

                def av(i):
                    rr = i - 4 * j
                    pt = pts.pop(i)
                    pt_inflight[0] -= 1
                    ntt = 4 - max(0, rr)
                    for h in range(2):
                        for tt in range(max(0, rr), 4):
                            nc.tensor.matmul(
                                yp[h][:, tt, 0:65],
                                lhsT=pt[:, h, 128 * tt:128 * (tt + 1)],
                                rhs=v[i][:, 2 * p + h, :],
                                start=(i == 0 and tt == 0),
                                stop=(i == 4 * j + tt),
                                skip_group_check=True,
                            )
                    pe_work(2 * ntt * 27.0)
                    # q-subtile tt's accumulation closed at chunk 4j+tt:
                    # stream its normalize (+ tail work) immediately
                    tt = i - 4 * j
                    if tt >= 0:
                        norm(tt)

                def norm(tt):
                    # ISA TensorScalar has no divide: per-head reciprocal of
                    # the denominator (col 0), then scalar-multiply.  In the
                    # tail the two heads' scales run on DVE and Pool in
                    # parallel (the chain is latency-critical there).
                    rc = p_yn.tile([128, 2], f32, name="rc", tag="rc", bufs=8)
                    ynst = p_yn.tile([128, 128], f16, name="ynst", tag="yn")
                    for h in range(2):
                        nc.vector.reciprocal(rc[:, h:h + 1], yp[h][:, tt, 0:1])
                        nc.vector.tensor_scalar(
                            ynst[:, 64 * h:64 * h + 64],
                            yp[h][:, tt, 1:65], rc[:, h:h + 1], None, mult)

                    def transpose_move(tt=tt, ynst=ynst):
                        tp = ps_mm.tile([128, 128], f16, name="tp", tag="mm")
                        nc.tensor.transpose(tp, ynst, ident)
                        nc.vector.tensor_copy(
                            ynT[p][:, 512 * j + 128 * tt:
                                   512 * j + 128 * (tt + 1)], tp)
                        pe_work(53.0)
                    if tail_hook is not None:
                        transpose_move()
                        tail_hook(tt)
                    else:
                        deferred.append(transpose_move)

                for i in range(ni):
                    if i not in pts:
                        maybe_emit_exp(force=(j, p, i))
                        assert i in pts
                    if i >= depth:
                        av(i - depth)
                    for _ in range(fps):
                        maybe_emit_exp()
                        filler()
                    maybe_emit_exp()
                for i in range(ni - depth, ni):
                    av(i)
                    maybe_emit_exp()
                    if tail_hook is None:
                        filler()
                        maybe_emit_exp()
                prerolled.pop((j, p), None)
                return deferred

            # ---- projection --------------------------------------------
            def proj_pieces(m):
                """output rows [128m, 128m+128) as 2 filler pieces; staging
                copies on DVE, full-row DMA via SWDGE at the end."""
                st = {"st": None}

                def pu(u):
                    if u == 0:
                        st["st"] = p_st.tile([128, 1024], f16, name="st_pr",
                                             tag="st")
                    ps = ps_mm.tile([128, 512], f32, name="ps_pr", tag="mm")
                    for kk in range(2):
                        nc.tensor.matmul(
                            ps[:, 0:512],
                            lhsT=ynT[kk][:, 128 * m:128 * (m + 1)],
                            rhs=wp[kk][:, 512 * u:512 * (u + 1)],
                            start=(kk == 0), stop=(kk == 1),
                        )
                    pe_work(2 * 213.0)
                    if exp_pos[0] >= len(exp_tasks):
                        nc.scalar.copy(
                            st["st"][:, 512 * u:512 * (u + 1)], ps[:, 0:512])
                    else:
                        nc.vector.tensor_copy(
                            st["st"][:, 512 * u:512 * (u + 1)], ps[:, 0:512])
                    if u == 1:
                        nc.sync.dma_start(
                            out=out_d[128 * m:128 * (m + 1), :], in_=st["st"])
                return [lambda: pu(0), lambda: pu(1)]

            def proj_tail(m, tt):
                """tail proj: dedicated ps_s halves, DVE+ACT copies in
                parallel, per-half DMAs fired as each copy lands."""
                st = p_st.tile([128, 1024], f16, name="st_pr", tag="st")
                ps = ps_s.tile([128, 2, 512], f32, name="ps_tl", tag="s")
                for u in range(2):
                    for kk in range(2):
                        nc.tensor.matmul(
                            ps[:, u, 0:512],
                            lhsT=ynT[kk][:, 128 * m:128 * (m + 1)],
                            rhs=wp[kk][:, 512 * u:512 * (u + 1)],
                            start=(kk == 0), stop=(kk == 1),
                        )
                pe_work(4 * 213.0)
                nc.scalar.copy(st[:, 0:512], ps[:, 0, 0:512])
                nc.sync.dma_start(out=out_d[128 * m:128 * (m + 1), 0:512],
                                  in_=st[:, 0:512])
                nc.vector.tensor_copy(st[:, 512:1024], ps[:, 1, 0:512])
                nc.scalar.dma_start(out=out_d[128 * m:128 * (m + 1), 512:1024],
                                    in_=st[:, 512:1024])

            # ---- filler machinery ---------------------------------------
            waves = []
            ordered = []

            def filler():
                if waves:
                    waves.pop(0)()
                elif ordered:
                    ordered.pop(0)()

            def drain(lst):
                while lst:
                    lst.pop(0)()

            # ---- emission order (scheduling priority) -------------------
            # ramp: kk-major over the [x8q1[k], w8qk[k]] DMA arrival stream.
            # ps_a accumulates the full q(0,0) chunk and ps_b the full k(2,0)
            # chunk; the ~1.28us of real matmul work per kk matches the
            # ~1.27us DMA cadence, so no warm filler is needed inside the
            # loop and the first S/exp can fire the moment the moves land.
            ps_a = ps_mm.tile([128, 512], f32, name="ps_qkv", tag="mm")
            ps_b = ps_mm.tile([128, 512], f32, name="ps_qkv", tag="mm")
            for kk in range(KT8):
                for ps, mi in ((ps_b, 2), (ps_a, 0)):
                    for g in range(3):
                        xa = xr8 if g == 2 else x8
                        nc.tensor.matmul(
                            ps[:, 0:512],
                            lhsT=w_qk(kk, mi, g),
                            rhs=xa[kk][:, :, 0:512],
                            start=(kk == 0 and g == 0),
                            stop=(kk == KT8 - 1 and g == 2),
                            perf_mode=DR,
                        )
                pe_work(6 * 213.0)
            qkv_move(ps_b, 2, 0)
            # q-move on ACT (Identity w/ scale+bias) in parallel with the
            # DVE k-move: the first S needs both
            nc.scalar.activation(qT[0][:, 0:512], ps_a[:, 0:512], Ident,
                                 bias=bqk[:, 0:1], scale=1.0 / WS)
            qdone.add((0, 0))
            # (1,0) pieces keep PE dense while the DVE moves land; the first
            # S/exp chunks slot in between.
            pc = qkv_pieces(1, 0)
            pc[0]()
            maybe_emit_exp()
            pc[1]()
            maybe_emit_exp()
            pc = qkv_pieces(3, 0)
            pc[0]()
            maybe_emit_exp()
            pc[1]()
            maybe_emit_exp()

            # PH0: pace the remaining (0,*) exps against v(0..3) + the j=1
            # qkv waves.  Pair-0 chunks first: they unlock (1,0,*) exps.
            ph0 = []
            for mi in (0, 2, 1, 3):
                ph0.extend(qkv_pieces(mi, 1))
            for m_ in range(4):
                ph0.extend(v_pieces(m_))
            for piece in ph0:
                piece()
                maybe_emit_exp()

            # waves for round j carry the j+2 qkv chunks (the global exp
            # stream runs ~2 rounds ahead) and the j+1 v chunks.
            # ordered: transposes + projs -- span boundaries.
            for j in range(NJ):
                if j < 2:
                    for mi in (0, 2, 1, 3):
                        waves.extend(qkv_pieces(mi, j + 2))
                for m_ in range(4 * j + 4, 4 * j + 8):
                    if m_ < NT:
                        waves.extend(v_pieces(m_))
                if j == 2:
                    for m_ in range(0, 4):
                        ordered.extend(proj_pieces(m_))
                elif j == 3:
                    for m_ in range(4, 12):
                        ordered.extend(proj_pieces(m_))
                for p in range(2):
                    if (j, p) == (3, 1):
                        def tail_hook(tt):
                            drain(ordered)
                            proj_tail(12 + tt, tt)
                        attention(j, p, filler, tail_hook=tail_hook)
                    else:
                        deferred = attention(j, p, filler, depth=2,
                                             fps=(2 if (j, p) == (3, 0) else 1))
                        ordered.extend(deferred)
                drain(waves)

    nc.compile()
    return nc


def _host_inputs(x, W_attn, b_attn, W_proj):
    """Build the 8 per-core input maps (numpy only)."""
    import ml_dtypes
    f8 = ml_dtypes.float8_e4m3

    x = np.asarray(x, dtype=np.float32)
    W_attn = np.asarray(W_attn, dtype=np.float32)
    b_attn = np.asarray(b_attn, dtype=np.float32)
    W_proj = np.asarray(W_proj, dtype=np.float32)

    # strict causal 0/1 mask for the 128x128 diagonal blocks: valid iff c >= k
    kl = np.arange(128)
    tri = (kl[None, :] >= kl[:, None]).astype(np.float16)
    ident = np.eye(128, dtype=np.float16)

    def pack8(a):
        """[C, N] -> fp8 main/residual tiles [KT8, 128, 2, N] each."""
        a8 = a.astype(f8)
        ar8 = (a - a8.astype(np.float32)).astype(f8)
        def t(z):
            return z.reshape(KT8, 2, 128, a.shape[1]).transpose(0, 2, 1, 3)
        return t(a8), t(ar8)

    in_maps = []
    for c in range(NCORES):
        b, g = divmod(c, 4)
        sl = slice(CH * g, CH * (g + 1))
        wq = W_attn[:, 0 * C:1 * C][:, sl] * SCALE
        wk = W_attn[:, 1 * C:2 * C][:, sl]
        wv = W_attn[:, 2 * C:3 * C][:, sl]
        bq = b_attn[0 * C:1 * C][sl] * SCALE
        bk = b_attn[1 * C:2 * C][sl]
        bv = b_attn[2 * C:3 * C][sl]
        bqk = np.stack([bq[0:128], bq[128:256], bk[0:128], bk[128:256]], axis=1)
        wfull = np.concatenate([wq, wk, wv], axis=1) * WS     # [1024, 768]
        w8, wr8 = pack8(wfull)
        xT = np.ascontiguousarray(x[b].T)                     # [1024, 2048]
        x8, xr8 = pack8(xT)
        # columns: [p0: q0m k0m q0r k0r | p1: q1m k1m q1r k1r | vm | vr]
        w8c = np.ascontiguousarray(np.concatenate(
            [w8[..., 0:128], w8[..., 256:384],
             wr8[..., 0:128], wr8[..., 256:384],
             w8[..., 128:256], w8[..., 384:512],
             wr8[..., 128:256], wr8[..., 384:512],
             w8[..., 512:768], wr8[..., 512:768]], axis=3))
        x8c = np.ascontiguousarray(np.stack([x8, xr8], axis=3))
        in_maps.append({
            "x8c": x8c, "w8c": w8c,
            "wp": np.ascontiguousarray(
                W_proj[sl, :].reshape(2, 128, C).astype(np.float16)),
            "bqk": np.ascontiguousarray(bqk),
            "bv": np.ascontiguousarray(
                np.broadcast_to(bv[None, :], (128, CH))),
            "tri": tri, "ident": ident,
        })
    return in_maps


def kernel(x, W_attn, b_attn, W_proj, b_proj, _want_results=None):
    global _COMPILED
    from concourse.bass_utils import run_bass_kernel_spmd

    if _COMPILED is None:
        _COMPILED = _build()
    nc = _COMPILED

    in_maps = _host_inputs(x, W_attn, b_attn, W_proj)
    kw = dict(_want_results or {})
    res = run_bass_kernel_spmd(nc, in_maps, core_ids=list(range(NCORES)), **kw)
    if _want_results is not None:
        kernel.last_results = res

    out = np.zeros((B, T, C), dtype=np.float32)
    for c in range(NCORES):
        out[c // 4] += res.results[c]["out_p"].astype(np.float32)
    out += np.asarray(b_proj, dtype=np.float32)[None, None, :]
    return out
